# revision 2
# baseline (speedup 1.0000x reference)
"""Trainium2 Bass kernel for nn_AttentionBlock (Swin-style 7x7 window attention).

Sharding: pure data parallel - batch B=8, one image per NeuronCore; small
weights and the 169x4 relative-bias table replicated (host-folded).

Per-core program (one image, built with Bass/Tile):
- Token order: window-compact, s = 49*w + p; chunk = 128 windows = 6272 tokens.
- Phase A: LN1 (token-major, window-on-partition) -> PE-transpose ->
  feature-major QKV (bf16 matmuls) -> per-window scores with PSUM-accumulated
  relative bias (pad columns = -1e30 so exp()=0) -> ACT exp -> AV matmuls with
  a ones-augmented V (sumexp lands per-token) -> normalize (DVE reciprocal +
  broadcast multiply) -> PE-transpose -> w_out -> residual -> y to DRAM.
- Phase B: LN2 -> FFN (gelu on ACT) -> residual -> scatter back to image order.

Self-contained: shapes/strategy hardcoded; only library imports.
"""
import numpy as np
import ml_dtypes

_CTX = {}
LAST_EXEC_NS = None

B = 8
N = 50176
D = 96
H = 4
DH = 32
HID = 384
EPS = 1e-5
W = 7
SCALE = DH ** -0.5
NEG = -1e30
NCHUNK = 8


def _rel_idx():
    pos = np.arange(W)
    gi, gj = np.meshgrid(pos, pos, indexing="ij")
    grid = np.stack([gi, gj], -1).reshape(-1, 2)
    rel = grid[:, None] - grid[None] + (W - 1)
    return rel[..., 0] * (2 * W - 1) + rel[..., 1]


def _host_consts(w_qkv, w_out, b_out, rel_bias, ln1_g, ln1_b, ln2_g, ln2_b,
                 w1, b1, w2, b2):
    bf = ml_dtypes.bfloat16
    w_qkv = np.asarray(w_qkv, np.float32)
    wq, wk, wv = w_qkv[0:128], w_qkv[128:256], w_qkv[256:384]
    g1 = np.asarray(ln1_g, np.float32)
    b1v = np.asarray(ln1_b, np.float32)

    def aug(wmat, gamma, beta, extra_scale=1.0):
        out = np.zeros((97, wmat.shape[0]), np.float32)
        out[0:96] = (wmat * gamma[None, :] * extra_scale).T
        out[96] = (wmat * extra_scale) @ beta
        return out

    wqT = aug(wq, g1, b1v, SCALE)
    wkT = aug(wk, g1, b1v)
    wvT_c = aug(wv, g1, b1v)
    wv_augT = np.zeros((97, 132), np.float32)
    for h in range(H):
        wv_augT[:, 33 * h:33 * h + 32] = wvT_c[:, 32 * h:32 * h + 32]
        wv_augT[96, 33 * h + 32] = 1.0
    rb = np.asarray(rel_bias, np.float32)
    bias_h = rb[_rel_idx()].transpose(2, 0, 1) * SCALE
    bias_tbl = np.full((49, H, 64), NEG, np.float32)
    for h in range(H):
        bias_tbl[:, h, 0:49] = bias_h[h]
    g2 = np.asarray(ln2_g, np.float32)
    b2v = np.asarray(ln2_b, np.float32)
    w1m = np.asarray(w1, np.float32)
    w1_augT = np.zeros((97, HID), np.float32)
    w1_augT[0:96] = (w1m * g2[None, :]).T
    w1_augT[96] = w1m @ b2v + np.asarray(b1, np.float32)
    w2T = np.asarray(w2, np.float32).T.reshape(3, 128, 96).transpose(1, 0, 2)
    c = {
        "wqT": wqT, "wkT": wkT, "wv_augT": wv_augT,
        "bias_tbl": bias_tbl, "eye49": np.eye(49, dtype=np.float32),
        "w_outT": np.asarray(w_out, np.float32).T,
        "b_out": np.asarray(b_out, np.float32).reshape(96, 1),
        "w1_augT": w1_augT, "w2T": w2T,
        "b2": np.asarray(b2, np.float32).reshape(96, 1),
        "ones_row": np.ones((1, 8192), np.float32),
    }
    return {k: (v.astype(np.float32) if k in ("b_out", "b2") else v.astype(bf))
            for k, v in c.items()}


def _split_multiwaits(nc, max_waits=1):
    """Walrus here allows 1 sync-wait per instruction; Tile emits multi-wait
    instructions. Split extras onto same-engine nops inserted just before."""
    import bass_rust

    def make_nop(eng):
        if hasattr(eng, "nop"):
            try:
                bi = eng.nop()
                return bi.ins if hasattr(bi, "ins") else bi
            except (AttributeError, TypeError):
                pass
        bi = eng.engine_nop()
        return bi.ins if hasattr(bi, "ins") else bi

    engines = {}
    for name in ("tensor", "vector", "scalar", "gpsimd", "sync"):
        eng = getattr(nc, name)
        engines[eng.engine] = eng
    n_split = 0
    for bbname, bbw in list(nc.bb_map.items()):
        lst = bbw.bb.instructions
        k = 0
        while k < len(lst):
            inst = lst[k]
            si = inst.sync_info
            if si is None:
                k += 1
                continue
            waits = list(si.on_wait or [])
            if len(waits) > max_waits:
                eng = engines.get(inst.engine)
                if eng is None:
                    k += 1
                    continue
                extra, keep = waits[:-max_waits], waits[-max_waits:]
                nops = []
                for w in extra:
                    nop_inst = make_nop(eng)
                    for obbw in nc.bb_map.values():
                        ol = obbw.bb.instructions
                        removed = False
                        for j in range(len(ol) - 1, -1, -1):
                            if ol[j] is nop_inst:
                                ol.pop(j)
                                removed = True
                                break
                        if removed:
                            break
                    nop_inst.sync_info = bass_rust.SyncInfo(on_wait=[w],
                                                            on_update=[])
                    nops.append(nop_inst)
                si.on_wait = keep
                inst.sync_info = si
                for j, nop_inst in enumerate(nops):
                    lst.insert(k + j, nop_inst)
                k += len(nops)
                n_split += 1
            k += 1
    return n_split


def _build(weights, n_chunks=NCHUNK):
    import concourse.bass as bass
    import concourse.tile as tile
    from concourse import mybir
    from concourse.masks import make_identity

    F32 = mybir.dt.float32
    BF16 = mybir.dt.bfloat16
    AF = mybir.ActivationFunctionType
    OP = mybir.AluOpType

    consts = _host_consts(**weights)
    nc = bass.Bass()
    xin = nc.declare_dram_parameter("x", [N, D], F32, isOutput=False)
    out_d = nc.declare_dram_parameter("out", [N, D], F32, isOutput=True)
    y_d = nc.dram_tensor("y_buf", [1024, 49, D], F32)
    cd = {}
    for k, v in consts.items():
        dt = F32 if v.dtype == np.float32 else BF16
        cd[k] = nc.declare_dram_parameter(k, list(v.shape), dt, isOutput=False)

    def dram_win_ap(t, chunk, wr_l):
        wr = 4 * chunk + wr_l
        base = (224 * 7 * wr) * 96
        return bass.AP(tensor=t, offset=base,
                       ap=[[7 * 96, 32], [224 * 96, 7], [1, 7 * 96]])

    with tile.TileContext(nc) as tc:
        pools = []

        def pool(name, bufs, space="SBUF"):
            p = tc.alloc_tile_pool(name=name, bufs=bufs, space=space)
            pools.append(p)
            return p

        kp = pool("konst", 1)
        sb_c = {}
        for k, darr in cd.items():
            if k == "ones_row":
                continue
            t = kp.tile(list(darr.shape), darr.dtype, name=f"c_{k}",
                        tag=f"c_{k}")
            nc.sync.dma_start(out=t,
                              in_=darr[(slice(None),) * len(darr.shape)])
            sb_c[k] = t
        ident = kp.tile([128, 128], BF16, name="ident", tag="ident")
        make_identity(nc, ident)

        apply_eng = nc.gpsimd if hasattr(nc.gpsimd, "tensor_scalar") \
            else nc.vector

        xp = pool("xp", 2)
        statp = pool("statp", 2)
        lnp = pool("lnp", 1)
        ltp = pool("ltp", 1)
        qkp = pool("qkp", 1)
        vap = pool("vap", 8)
        exp_p = pool("exp_p", 4)
        onp = pool("onp", 4)
        otp = pool("otp", 4)
        aop = pool("aop", 1)
        yp = pool("yp", 1)
        h1p = pool("h1p", 3)
        fop = pool("fop", 1)
        outp = pool("outp", 1)

        ps_s = pool("ps_s", 1, space="PSUM")
        ps_a = pool("ps_a", 2, space="PSUM")
        ps_m = pool("ps_m", 1, space="PSUM")
        ps_t = pool("ps_t", 1, space="PSUM")

        def ln_layer(src, name):
            stats = statp.tile([128, 49, 6], F32, name=f"st_{name}", tag="st")
            mv = statp.tile([128, 49, 2], F32, name=f"mv_{name}", tag="mv")
            for p in range(49):
                nc.vector.bn_stats(out=stats[:, p, :], in_=src[:, p, :])
                nc.vector.bn_aggr(out=mv[:, p, :], in_=stats[:, p, :])
            veps = statp.tile([128, 49], F32, name=f"ve_{name}", tag="veps")
            nc.vector.tensor_scalar_add(veps, mv[:, :, 1], EPS)
            lnv = statp.tile([128, 49], F32, name=f"lv_{name}", tag="lnv")
            nc.scalar.activation(out=lnv, in_=veps, func=AF.Ln)
            rstd = statp.tile([128, 49], F32, name=f"rs_{name}", tag="rstd")
            nc.scalar.activation(out=rstd, in_=lnv, func=AF.Exp, scale=-0.5)
            lnt = lnp.tile([128, 49, 96], BF16, name=f"ln_{name}", tag="lnt")
            for p in range(49):
                apply_eng.tensor_scalar(
                    out=lnt[:, p, :], in0=src[:, p, :],
                    scalar1=mv[:, p, 0:1], op0=OP.subtract,
                    scalar2=rstd[:, p:p + 1], op1=OP.mult)
            return lnt

        def transpose_to_T(lnt, dstT):
            for p0 in range(0, 49, 4):
                k = min(4, 49 - p0)
                tp = ps_t.tile([128, 512], BF16, name="tp", tag="tps")
                for j in range(k):
                    nc.tensor.transpose(tp[0:96, 128 * j:128 * j + 128],
                                        lnt[:, p0 + j, :], ident)
                nc.vector.tensor_copy(
                    out=dstT[0:96, 128 * p0:128 * p0 + 128 * k],
                    in_=tp[0:96, 0:128 * k])

        def transpose_to_tok_add(srcT, add_t, dst_t):
            for p0 in range(0, 49, 4):
                k = min(4, 49 - p0)
                tp = ps_t.tile([128, 512], BF16, name="tt", tag="tps")
                for j in range(k):
                    p = p0 + j
                    nc.tensor.transpose(tp[0:128, 96 * j:96 * j + 96],
                                        srcT[0:96, 128 * p:128 * p + 128],
                                        ident[0:96, 0:96])
                nc.vector.scalar_tensor_tensor(
                    out=dst_t[:, p0:p0 + k, :],
                    in0=tp[0:128, 0:96 * k].rearrange("a (b c) -> a b c",
                                                      c=96),
                    scalar=1.0, op0=OP.mult,
                    in1=add_t[:, p0:p0 + k, :], op1=OP.add)

        eye_b = sb_c["eye49"][:, None, :].to_broadcast([49, 8, 49])

        # ---------------- PHASE A ----------------
        for c in range(n_chunks):
            x_tok = xp.tile([128, 49, 96], F32, name="x_tok", tag="x_tok")
            for wr_l in range(4):
                nc.sync.dma_start(out=x_tok[32 * wr_l:32 * wr_l + 32, :, :],
                                  in_=dram_win_ap(xin, c, wr_l))

            ln_x = ln_layer(x_tok, f"a{c}")
            ln_xT = ltp.tile([128, 8192], BF16, name="ln_xT", tag="ln_xT")
            nc.gpsimd.memset(ln_xT[0:97, 6272:8192], 0.0)
            nc.sync.dma_start(out=ln_xT[96:97, 0:6272],
                              in_=cd["ones_row"][0:1, 0:6272])
            transpose_to_T(ln_x, ln_xT)
            ln_xT_w = ln_xT.rearrange("k (p w) -> k w p", w=128)

            qT = qkp.tile([128, 6272], BF16, name="qT", tag="qT")
            kT = qkp.tile([128, 6272], BF16, name="kT", tag="kT")
            for n0 in range(0, 6272, 448):
                qps = ps_m.tile([128, 448], F32, name="qps", tag="mm")
                nc.tensor.matmul(qps, sb_c["wqT"][0:97, :],
                                 ln_xT[0:97, n0:n0 + 448],
                                 start=True, stop=True)
                nc.vector.tensor_copy(out=qT[:, n0:n0 + 448], in_=qps)
                kps = ps_m.tile([128, 448], F32, name="kps", tag="mm")
                nc.tensor.matmul(kps, sb_c["wkT"][0:97, :],
                                 ln_xT[0:97, n0:n0 + 448],
                                 start=True, stop=True)
                nc.scalar.copy(out=kT[:, n0:n0 + 448], in_=kps)

            qT_w = qT.rearrange("k (p w) -> k w p", w=128)
            kT_w = kT.rearrange("k (p w) -> k w p", w=128)

            attn_T = aop.tile([96, 6272], BF16, name="attn_T", tag="attn_T")
            attn_T_w = attn_T.rearrange("a (p w) -> a p w", w=128)

            for g in range(8):
                w0 = 16 * g
                v_tiles = []
                for q2 in range(4):
                    vps = ps_a.tile([128, 264], F32, name="vps", tag="av")
                    for dpq in range(2):
                        for par in range(2):
                            wloc = w0 + 2 * (2 * q2 + dpq) + par
                            nc.tensor.matmul(
                                vps[64 * par:64 * par + 64,
                                    132 * dpq:132 * dpq + 132],
                                ln_xT_w[0:97, wloc, :],
                                sb_c["wv_augT"][0:97, :],
                                start=True, stop=True, skip_group_check=True)
                    v_sb = vap.tile([128, 264], BF16, name="v_sb", tag="v_sb")
                    nc.scalar.copy(out=v_sb, in_=vps)
                    v_tiles.append(v_sb)

                exp_ts = []
                for hh in (0, 2):
                    sps = ps_s.tile([128, 1024], F32, name="sps", tag="sps")
                    sps_r = sps.rearrange("a (r x) -> a r x", x=512)
                    for r in range(2):
                        h = hh + r
                        for par in range(2):
                            nc.tensor.matmul(
                                sps_r[64 * par:64 * par + 64, r, 0:392],
                                sb_c["bias_tbl"][:, h, :], eye_b,
                                start=True, stop=False, skip_group_check=True)
                        for j in range(8):
                            for par in range(2):
                                wloc = w0 + 2 * j + par
                                nc.tensor.matmul(
                                    sps_r[64 * par:64 * par + 49, r,
                                          49 * j:49 * j + 49],
                                    kT_w[32 * h:32 * h + 32, wloc, 0:49],
                                    qT_w[32 * h:32 * h + 32, wloc, 0:49],
                                    start=False, stop=True,
                                    tile_position=(32 * h, 64 * par),
                                    skip_group_check=True)
                    exp_sb = exp_p.tile([128, 2, 392], BF16, name="exp_sb",
                                        tag="exp_sb")
                    nc.scalar.activation(out=exp_sb, in_=sps_r[:, :, 0:392],
                                         func=AF.Exp)
                    exp_ts.append(exp_sb)

                for q2 in range(4):
                    avp = ps_a.tile([128, 264], F32, name="avp", tag="av")
                    avp_b = avp.rearrange("a (b c) -> a b c", c=33)
                    for dpq in range(2):
                        j = 2 * q2 + dpq
                        for h in range(4):
                            e_sb = exp_ts[h // 2]
                            r = h % 2
                            for par in range(2):
                                nc.tensor.matmul(
                                    avp[64 * par:64 * par + 49,
                                        132 * dpq + 33 * h:
                                        132 * dpq + 33 * h + 33],
                                    e_sb[64 * par:64 * par + 64, r,
                                         49 * j:49 * j + 49],
                                    v_tiles[q2][64 * par:64 * par + 64,
                                                132 * dpq + 33 * h:
                                                132 * dpq + 33 * h + 33],
                                    start=True, stop=True,
                                    skip_group_check=True)
                    rc = statp.tile([128, 8], F32, name="rc", tag="rc")
                    nc.vector.reciprocal(
                        rc.rearrange("a (b c) -> a b c", c=1),
                        avp_b[:, :, 32:33])
                    o_norm = onp.tile([128, 256], BF16, name="o_norm",
                                      tag="o_norm")
                    nc.vector.scalar_tensor_tensor(
                        out=o_norm.rearrange("a (b c) -> a b c", c=32),
                        in0=avp_b[:, :, 0:32], scalar=1.0, op0=OP.mult,
                        in1=rc[:, :, None].to_broadcast([128, 8, 32]),
                        op1=OP.mult)

                    otps = ps_t.tile([128, 256], BF16, name="otps", tag="tps")
                    for dpq in range(2):
                        nc.tensor.transpose(
                            otps[:, 128 * dpq:128 * dpq + 128],
                            o_norm[:, 128 * dpq:128 * dpq + 128], ident)
                    oT_sb = otp.tile([128, 256], BF16, name="oT_sb",
                                     tag="oT_sb")
                    nc.vector.tensor_copy(out=oT_sb, in_=otps)

                    wop = ps_m.tile([96, 196], F32, name="wop", tag="wout")
                    nc.tensor.matmul(
                        wop, sb_c["w_outT"],
                        oT_sb.rearrange("a (r m q) -> a r m q",
                                        r=2, q=64)[:, :, :, 0:49],
                        start=True, stop=True)
                    wbase = 2 * (8 * g + 2 * q2)
                    nc.scalar.activation(
                        out=attn_T_w[:, :, wbase:wbase + 4].transpose(
                            [0, 2, 1]),
                        in_=wop.rearrange("a (r m q) -> a (r m) q",
                                          r=2, m=2),
                        func=AF.Identity, bias=sb_c["b_out"][0:96, 0:1],
                        scale=1.0)

            y_tok = yp.tile([128, 49, 96], F32, name="y_tok", tag="y_tok")
            transpose_to_tok_add(attn_T, x_tok, y_tok)
            nc.sync.dma_start(out=y_d[128 * c:128 * c + 128, :, :], in_=y_tok)

        # ---------------- PHASE B ----------------
        for c in range(n_chunks):
            y_in = xp.tile([128, 49, 96], F32, name="y_in", tag="x_tok")
            nc.sync.dma_start(out=y_in, in_=y_d[128 * c:128 * c + 128, :, :])

            ln2 = ln_layer(y_in, f"b{c}")
            ln2T = ltp.tile([128, 6272], BF16, name="ln2T", tag="ln2T")
            nc.sync.dma_start(out=ln2T[96:97, 0:6272],
                              in_=cd["ones_row"][0:1, 0:6272])
            transpose_to_T(ln2, ln2T)

            ffn_T = fop.tile([96, 6272], BF16, name="ffn_T", tag="ffn_T")
            for n0 in range(0, 6272, 448):
                h1 = h1p.tile([128, 3, 448], BF16, name="h1", tag="h1")
                for m in range(3):
                    fps = ps_m.tile([128, 448], F32, name="fps", tag="mm")
                    nc.tensor.matmul(
                        fps, sb_c["w1_augT"][0:97, 128 * m:128 * m + 128],
                        ln2T[0:97, n0:n0 + 448], start=True, stop=True)
                    nc.scalar.activation(out=h1[:, m, :], in_=fps,
                                         func=AF.Gelu)
                f2 = ps_m.tile([96, 448], F32, name="f2", tag="mm2")
                for m in range(3):
                    nc.tensor.matmul(f2, sb_c["w2T"][:, m, :], h1[:, m, :],
                                     start=(m == 0), stop=(m == 2),
                                     skip_group_check=True)
                nc.scalar.activation(out=ffn_T[0:96, n0:n0 + 448], in_=f2,
                                     func=AF.Identity,
                                     bias=sb_c["b2"][0:96, 0:1], scale=1.0)

            out_tok = outp.tile([128, 49, 96], F32, name="out_tok",
                                tag="out_tok")
            transpose_to_tok_add(ffn_T, y_in, out_tok)
            for wr_l in range(4):
                nc.sync.dma_start(out=dram_win_ap(out_d, c, wr_l),
                                  in_=out_tok[32 * wr_l:32 * wr_l + 32, :, :])

        for p in reversed(pools):
            p.release()

    _split_multiwaits(nc)
    return nc, consts


def kernel(x, w_qkv, w_out, b_out, rel_bias, ln1_g, ln1_b, ln2_g, ln2_b,
           w1, b1, w2, b2):
    from concourse.bass_utils import run_bass_kernel_spmd

    if "nc" not in _CTX:
        weights = dict(w_qkv=w_qkv, w_out=w_out, b_out=b_out,
                       rel_bias=rel_bias, ln1_g=ln1_g, ln1_b=ln1_b,
                       ln2_g=ln2_g, ln2_b=ln2_b, w1=w1, b1=b1, w2=w2, b2=b2)
        nc, consts = _build(weights)
        _CTX["nc"] = nc
        _CTX["consts"] = consts

    x = np.asarray(x, np.float32)
    in_maps = []
    for i in range(B):
        m = {"x": np.ascontiguousarray(x[i])}
        m.update(_CTX["consts"])
        in_maps.append(m)
    _CTX["in_maps"] = in_maps

    res = run_bass_kernel_spmd(_CTX["nc"], in_maps, core_ids=list(range(B)))
    global LAST_EXEC_NS
    if res.exec_time_ns:
        LAST_EXEC_NS = res.exec_time_ns
    out = np.stack([np.asarray(res.results[i]["out"]) for i in range(B)])
    return out.astype(np.float32)


# revision 4
# speedup vs baseline: 1.1544x; 1.1544x over previous
"""Trainium2 Bass kernel for nn_AttentionBlock (Swin-style 7x7 window attention).

Sharding: pure data parallel - batch B=8, one image per NeuronCore; small
weights and the 169x4 relative-bias table replicated (host-folded).

Per-core program (one image, built with Bass/Tile):
- Token order: window-compact, s = 49*w + p; chunk = 128 windows = 6272 tokens.
- Phase A: LN1 (token-major, window-on-partition) -> PE-transpose ->
  feature-major QKV (bf16 matmuls) -> per-window scores with PSUM-accumulated
  relative bias (pad columns = -1e30 so exp()=0) -> ACT exp -> AV matmuls with
  a ones-augmented V (sumexp lands per-token) -> normalize (DVE reciprocal +
  broadcast multiply) -> PE-transpose -> w_out -> residual -> y to DRAM.
- Phase B: LN2 -> FFN (gelu on ACT) -> residual -> scatter back to image order.

Self-contained: shapes/strategy hardcoded; only library imports.
"""
import numpy as np
import ml_dtypes

_CTX = {}
LAST_EXEC_NS = None

B = 8
N = 50176
D = 96
H = 4
DH = 32
HID = 384
EPS = 1e-5
W = 7
SCALE = DH ** -0.5
NEG = -1e30
NCHUNK = 8


def _rel_idx():
    pos = np.arange(W)
    gi, gj = np.meshgrid(pos, pos, indexing="ij")
    grid = np.stack([gi, gj], -1).reshape(-1, 2)
    rel = grid[:, None] - grid[None] + (W - 1)
    return rel[..., 0] * (2 * W - 1) + rel[..., 1]


def _host_consts(w_qkv, w_out, b_out, rel_bias, ln1_g, ln1_b, ln2_g, ln2_b,
                 w1, b1, w2, b2):
    bf = ml_dtypes.bfloat16
    w_qkv = np.asarray(w_qkv, np.float32)
    wq, wk, wv = w_qkv[0:128], w_qkv[128:256], w_qkv[256:384]
    g1 = np.asarray(ln1_g, np.float32)
    b1v = np.asarray(ln1_b, np.float32)

    def aug(wmat, gamma, beta, extra_scale=1.0):
        out = np.zeros((97, wmat.shape[0]), np.float32)
        out[0:96] = (wmat * gamma[None, :] * extra_scale).T
        out[96] = (wmat * extra_scale) @ beta
        return out

    wqT = aug(wq, g1, b1v, SCALE)
    wkT = aug(wk, g1, b1v)
    wvT_c = aug(wv, g1, b1v)
    wv_augT = np.zeros((97, 132), np.float32)
    for h in range(H):
        wv_augT[:, 33 * h:33 * h + 32] = wvT_c[:, 32 * h:32 * h + 32]
        wv_augT[96, 33 * h + 32] = 1.0
    rb = np.asarray(rel_bias, np.float32)
    bias_h = rb[_rel_idx()].transpose(2, 0, 1) * SCALE
    bias_tbl = np.full((49, H, 64), NEG, np.float32)
    for h in range(H):
        bias_tbl[:, h, 0:49] = bias_h[h]
    g2 = np.asarray(ln2_g, np.float32)
    b2v = np.asarray(ln2_b, np.float32)
    w1m = np.asarray(w1, np.float32)
    w1_augT = np.zeros((97, HID), np.float32)
    w1_augT[0:96] = (w1m * g2[None, :]).T
    w1_augT[96] = w1m @ b2v + np.asarray(b1, np.float32)
    w2T = np.asarray(w2, np.float32).T.reshape(3, 128, 96).transpose(1, 0, 2)
    c = {
        "wqT": wqT, "wkT": wkT, "wv_augT": wv_augT,
        "bias_tbl": bias_tbl, "eye49": np.eye(49, dtype=np.float32),
        "w_outT": np.asarray(w_out, np.float32).T,
        "b_out": np.asarray(b_out, np.float32).reshape(96, 1),
        "w1_augT": w1_augT, "w2T": w2T,
        "b2": np.asarray(b2, np.float32).reshape(96, 1),
        "ones_row": np.ones((1, 8192), np.float32),
    }
    return {k: (v.astype(np.float32) if k in ("b_out", "b2") else v.astype(bf))
            for k, v in c.items()}


def _split_multiwaits(nc, max_waits=1):
    """Walrus here allows 1 sync-wait per instruction; Tile emits multi-wait
    instructions. Split extras onto same-engine nops inserted just before."""
    import bass_rust

    def make_nop(eng):
        if hasattr(eng, "nop"):
            try:
                bi = eng.nop()
                return bi.ins if hasattr(bi, "ins") else bi
            except (AttributeError, TypeError):
                pass
        bi = eng.engine_nop()
        return bi.ins if hasattr(bi, "ins") else bi

    engines = {}
    for name in ("tensor", "vector", "scalar", "gpsimd", "sync"):
        eng = getattr(nc, name)
        engines[eng.engine] = eng
    n_split = 0
    for bbname, bbw in list(nc.bb_map.items()):
        lst = bbw.bb.instructions
        k = 0
        while k < len(lst):
            inst = lst[k]
            si = inst.sync_info
            if si is None:
                k += 1
                continue
            waits = list(si.on_wait or [])
            if len(waits) > max_waits:
                eng = engines.get(inst.engine)
                if eng is None:
                    k += 1
                    continue
                extra, keep = waits[:-max_waits], waits[-max_waits:]
                nops = []
                for w in extra:
                    nop_inst = make_nop(eng)
                    for obbw in nc.bb_map.values():
                        ol = obbw.bb.instructions
                        removed = False
                        for j in range(len(ol) - 1, -1, -1):
                            if ol[j] is nop_inst:
                                ol.pop(j)
                                removed = True
                                break
                        if removed:
                            break
                    nop_inst.sync_info = bass_rust.SyncInfo(on_wait=[w],
                                                            on_update=[])
                    nops.append(nop_inst)
                si.on_wait = keep
                inst.sync_info = si
                for j, nop_inst in enumerate(nops):
                    lst.insert(k + j, nop_inst)
                k += len(nops)
                n_split += 1
            k += 1
    return n_split


def _build(weights, n_chunks=NCHUNK):
    import concourse.bass as bass
    import concourse.tile as tile
    from concourse import mybir
    from concourse.masks import make_identity

    F32 = mybir.dt.float32
    BF16 = mybir.dt.bfloat16
    AF = mybir.ActivationFunctionType
    OP = mybir.AluOpType
    AX = mybir.AxisListType

    consts = _host_consts(**weights)
    nc = bass.Bass()
    xin = nc.declare_dram_parameter("x", [N, D], F32, isOutput=False)
    out_d = nc.declare_dram_parameter("out", [N, D], F32, isOutput=True)
    y_d = nc.dram_tensor("y_buf", [1024, 49, D], F32)
    cd = {}
    for k, v in consts.items():
        dt = F32 if v.dtype == np.float32 else BF16
        cd[k] = nc.declare_dram_parameter(k, list(v.shape), dt, isOutput=False)

    def dram_win_ap(t, chunk, wr_l):
        wr = 4 * chunk + wr_l
        base = (224 * 7 * wr) * 96
        return bass.AP(tensor=t, offset=base,
                       ap=[[7 * 96, 32], [224 * 96, 7], [1, 7 * 96]])

    with tile.TileContext(nc) as tc:
        pools = []

        def pool(name, bufs, space="SBUF"):
            p = tc.alloc_tile_pool(name=name, bufs=bufs, space=space)
            pools.append(p)
            return p

        kp = pool("konst", 1)
        sb_c = {}
        for k, darr in cd.items():
            if k == "ones_row":
                continue
            t = kp.tile(list(darr.shape), darr.dtype, name=f"c_{k}",
                        tag=f"c_{k}")
            nc.sync.dma_start(out=t,
                              in_=darr[(slice(None),) * len(darr.shape)])
            sb_c[k] = t
        ident = kp.tile([128, 128], BF16, name="ident", tag="ident")
        make_identity(nc, ident)

        xp = pool("xp", 2)
        statp = pool("statp", 2)
        lnp = pool("lnp", 1)
        ltp = pool("ltp", 1)
        qkp = pool("qkp", 1)
        vap = pool("vap", 6)
        exp_p = pool("exp_p", 3)
        onp = pool("onp", 2)
        otp = pool("otp", 2)
        aop = pool("aop", 1)
        yp = pool("yp", 1)
        tokp = pool("tokp", 1)
        h1p = pool("h1p", 3)
        fop = pool("fop", 1)
        outp = pool("outp", 1)

        ps_s = pool("ps_s", 1, space="PSUM")
        ps_a = pool("ps_a", 3, space="PSUM")
        ps_m = pool("ps_m", 1, space="PSUM")

        def ln_layer(src, name):
            """src [128, 49, 96] f32 -> lnt [128, 49, 128] bf16 (pad cols junk).

            Batched stats: one reduce for sums, one mul + reduce for sumsq.
            """
            sums = statp.tile([128, 49], F32, name=f"sm_{name}", tag="sums")
            nc.vector.tensor_reduce(out=sums, in_=src, axis=AX.X, op=OP.add)
            sq = lnp.tile([128, 49, 96], F32, name=f"sq_{name}", tag="lnt")
            nc.vector.tensor_mul(sq, src, src)
            sumsq = statp.tile([128, 49], F32, name=f"s2_{name}", tag="sumsq")
            nc.vector.tensor_reduce(out=sumsq, in_=sq, axis=AX.X, op=OP.add)
            m = statp.tile([128, 49], F32, name=f"m_{name}", tag="m")
            nc.vector.tensor_scalar_mul(m, sums, 1.0 / 96)
            # veps = sumsq/96 - m^2 + EPS
            m2 = statp.tile([128, 49], F32, name=f"m2_{name}", tag="m2")
            nc.vector.tensor_mul(m2, m, m)
            veps = statp.tile([128, 49], F32, name=f"ve_{name}", tag="veps")
            nc.vector.scalar_tensor_tensor(
                out=veps, in0=sumsq, scalar=1.0 / 96, op0=OP.mult,
                in1=m2, op1=OP.subtract)
            nc.vector.tensor_scalar_add(veps, veps, EPS)
            lnv = statp.tile([128, 49], F32, name=f"lv_{name}", tag="lnv")
            nc.scalar.activation(out=lnv, in_=veps, func=AF.Ln)
            rstd = statp.tile([128, 49], F32, name=f"rs_{name}", tag="rstd")
            nc.scalar.activation(out=rstd, in_=lnv, func=AF.Exp, scale=-0.5)
            lnt = lnp.tile([128, 49, 128], BF16, name=f"ln_{name}", tag="lnt")
            for p in range(49):
                nc.vector.tensor_scalar(
                    out=lnt[:, p, 0:96], in0=src[:, p, :],
                    scalar1=m[:, p:p + 1], op0=OP.subtract,
                    scalar2=rstd[:, p:p + 1], op1=OP.mult)
            return lnt

        def transpose_to_T(lnt, dstT):
            """DMA block-transpose: lnt [128, 49, 128] -> dstT[:, 0:6272]
            position-major (col = 128*p + w)."""
            nc.sync.dma_start_transpose(
                dstT[:, 0:6272].rearrange("a (p w) -> a p w", w=128),
                lnt.rearrange("a b c -> a (b c)"))

        def transpose_to_tok(srcT, dst_tok):
            """DMA block-transpose: srcT [96, 6272] -> dst_tok [128, 49, 96]."""
            nc.sync.dma_start_transpose(dst_tok, srcT[0:96, 0:6272])

        eye_b = sb_c["eye49"][:, None, :].to_broadcast([49, 8, 49])

        # ---------------- PHASE A ----------------
        for c in range(n_chunks):
            x_tok = xp.tile([128, 49, 96], F32, name="x_tok", tag="x_tok")
            for wr_l in range(4):
                nc.sync.dma_start(out=x_tok[32 * wr_l:32 * wr_l + 32, :, :],
                                  in_=dram_win_ap(xin, c, wr_l))

            ln_x = ln_layer(x_tok, f"a{c}")
            ln_xT = ltp.tile([128, 8192], BF16, name="ln_xT", tag="ln_xT")
            nc.gpsimd.memset(ln_xT[0:97, 6272:8192], 0.0)
            transpose_to_T(ln_x, ln_xT)
            nc.sync.dma_start(out=ln_xT[96:97, 0:6272],
                              in_=cd["ones_row"][0:1, 0:6272])
            ln_xT_w = ln_xT.rearrange("k (p w) -> k w p", w=128)

            qT = qkp.tile([128, 6272], BF16, name="qT", tag="qT")
            kT = qkp.tile([128, 6272], BF16, name="kT", tag="kT")
            for n0 in range(0, 6272, 448):
                qps = ps_m.tile([128, 448], F32, name="qps", tag="mm")
                nc.tensor.matmul(qps, sb_c["wqT"][0:97, :],
                                 ln_xT[0:97, n0:n0 + 448],
                                 start=True, stop=True)
                nc.vector.tensor_copy(out=qT[:, n0:n0 + 448], in_=qps)
                kps = ps_m.tile([128, 448], F32, name="kps", tag="mm")
                nc.tensor.matmul(kps, sb_c["wkT"][0:97, :],
                                 ln_xT[0:97, n0:n0 + 448],
                                 start=True, stop=True)
                nc.scalar.copy(out=kT[:, n0:n0 + 448], in_=kps)

            qT_w = qT.rearrange("k (p w) -> k w p", w=128)
            kT_w = kT.rearrange("k (p w) -> k w p", w=128)

            attn_T = aop.tile([96, 6272], BF16, name="attn_T", tag="attn_T")
            attn_T_w = attn_T.rearrange("a (p w) -> a p w", w=128)

            for g in range(8):
                w0 = 16 * g
                v_tiles = []
                for q2 in range(4):
                    vps = ps_a.tile([128, 264], F32, name="vps", tag="av")
                    for dpq in range(2):
                        for par in range(2):
                            wloc = w0 + 2 * (2 * q2 + dpq) + par
                            nc.tensor.matmul(
                                vps[64 * par:64 * par + 64,
                                    132 * dpq:132 * dpq + 132],
                                ln_xT_w[0:97, wloc, :],
                                sb_c["wv_augT"][0:97, :],
                                start=True, stop=True, skip_group_check=True)
                    v_sb = vap.tile([128, 264], BF16, name="v_sb", tag="v_sb")
                    nc.scalar.copy(out=v_sb, in_=vps)
                    v_tiles.append(v_sb)

                exp_ts = []
                for hh in (0, 2):
                    sps = ps_s.tile([128, 1024], F32, name="sps", tag="sps")
                    sps_r = sps.rearrange("a (r x) -> a r x", x=512)
                    for r in range(2):
                        h = hh + r
                        for par in range(2):
                            nc.tensor.matmul(
                                sps_r[64 * par:64 * par + 64, r, 0:392],
                                sb_c["bias_tbl"][:, h, :], eye_b,
                                start=True, stop=False, skip_group_check=True)
                        for j in range(8):
                            for par in range(2):
                                wloc = w0 + 2 * j + par
                                nc.tensor.matmul(
                                    sps_r[64 * par:64 * par + 49, r,
                                          49 * j:49 * j + 49],
                                    kT_w[32 * h:32 * h + 32, wloc, 0:49],
                                    qT_w[32 * h:32 * h + 32, wloc, 0:49],
                                    start=False, stop=True,
                                    tile_position=(32 * h, 64 * par),
                                    skip_group_check=True)
                    exp_sb = exp_p.tile([128, 2, 392], BF16, name="exp_sb",
                                        tag="exp_sb")
                    nc.scalar.activation(out=exp_sb, in_=sps_r[:, :, 0:392],
                                         func=AF.Exp)
                    exp_ts.append(exp_sb)

                for q2 in range(4):
                    avp = ps_a.tile([128, 264], F32, name="avp", tag="av")
                    avp_b = avp.rearrange("a (b c) -> a b c", c=33)
                    for dpq in range(2):
                        j = 2 * q2 + dpq
                        for h in range(4):
                            e_sb = exp_ts[h // 2]
                            r = h % 2
                            for par in range(2):
                                nc.tensor.matmul(
                                    avp[64 * par:64 * par + 49,
                                        132 * dpq + 33 * h:
                                        132 * dpq + 33 * h + 33],
                                    e_sb[64 * par:64 * par + 64, r,
                                         49 * j:49 * j + 49],
                                    v_tiles[q2][64 * par:64 * par + 64,
                                                132 * dpq + 33 * h:
                                                132 * dpq + 33 * h + 33],
                                    start=True, stop=True,
                                    skip_group_check=True)
                    rc = statp.tile([128, 8], F32, name="rc", tag="rc")
                    nc.vector.reciprocal(
                        rc.rearrange("a (b c) -> a b c", c=1),
                        avp_b[:, :, 32:33])
                    o_norm = onp.tile([128, 256], BF16, name="o_norm",
                                      tag="o_norm")
                    nc.vector.scalar_tensor_tensor(
                        out=o_norm.rearrange("a (b c) -> a b c", c=32),
                        in0=avp_b[:, :, 0:32], scalar=1.0, op0=OP.mult,
                        in1=rc[:, :, None].to_broadcast([128, 8, 32]),
                        op1=OP.mult)

                    oT_sb = otp.tile([128, 256], BF16, name="oT_sb",
                                     tag="oT_sb")
                    nc.sync.dma_start_transpose(
                        oT_sb.rearrange("a (r c) -> a r c", c=128),
                        o_norm[:, :])

                    wop = ps_m.tile([96, 196], F32, name="wop", tag="wout")
                    nc.tensor.matmul(
                        wop, sb_c["w_outT"],
                        oT_sb.rearrange("a (r m q) -> a r m q",
                                        r=2, q=64)[:, :, :, 0:49],
                        start=True, stop=True)
                    wbase = 2 * (8 * g + 2 * q2)
                    nc.vector.tensor_scalar_add(
                        attn_T_w[:, :, wbase:wbase + 4].transpose([0, 2, 1]),
                        wop.rearrange("a (r m q) -> a (r m) q", r=2, m=2),
                        sb_c["b_out"][0:96, 0:1])

            attn_tok = tokp.tile([128, 49, 96], BF16, name="attn_tok",
                                 tag="tok_b")
            transpose_to_tok(attn_T, attn_tok)
            y_tok = yp.tile([128, 49, 96], F32, name="y_tok", tag="y_tok")
            nc.vector.scalar_tensor_tensor(
                out=y_tok, in0=attn_tok, scalar=1.0, op0=OP.mult,
                in1=x_tok, op1=OP.add)
            nc.sync.dma_start(out=y_d[128 * c:128 * c + 128, :, :], in_=y_tok)

        # ---------------- PHASE B ----------------
        for c in range(n_chunks):
            y_in = xp.tile([128, 49, 96], F32, name="y_in", tag="x_tok")
            nc.sync.dma_start(out=y_in, in_=y_d[128 * c:128 * c + 128, :, :])

            ln2 = ln_layer(y_in, f"b{c}")
            ln2T = ltp.tile([128, 6272], BF16, name="ln2T", tag="ln2T")
            transpose_to_T(ln2, ln2T)
            nc.sync.dma_start(out=ln2T[96:97, 0:6272],
                              in_=cd["ones_row"][0:1, 0:6272])

            ffn_T = fop.tile([96, 6272], BF16, name="ffn_T", tag="ffn_T")
            for n0 in range(0, 6272, 448):
                h1 = h1p.tile([128, 3, 448], BF16, name="h1", tag="h1")
                for m in range(3):
                    fps = ps_m.tile([128, 448], F32, name="fps", tag="mm")
                    nc.tensor.matmul(
                        fps, sb_c["w1_augT"][0:97, 128 * m:128 * m + 128],
                        ln2T[0:97, n0:n0 + 448], start=True, stop=True)
                    nc.scalar.activation(out=h1[:, m, :], in_=fps,
                                         func=AF.Gelu)
                f2 = ps_m.tile([96, 448], F32, name="f2", tag="mm2")
                for m in range(3):
                    nc.tensor.matmul(f2, sb_c["w2T"][:, m, :], h1[:, m, :],
                                     start=(m == 0), stop=(m == 2),
                                     skip_group_check=True)
                nc.vector.tensor_scalar_add(ffn_T[0:96, n0:n0 + 448], f2,
                                             sb_c["b2"][0:96, 0:1])

            ffn_tok = tokp.tile([128, 49, 96], BF16, name="ffn_tok",
                                tag="tok_b")
            transpose_to_tok(ffn_T, ffn_tok)
            out_tok = outp.tile([128, 49, 96], F32, name="out_tok",
                                tag="out_tok")
            nc.vector.scalar_tensor_tensor(
                out=out_tok, in0=ffn_tok, scalar=1.0, op0=OP.mult,
                in1=y_in, op1=OP.add)
            for wr_l in range(4):
                nc.sync.dma_start(out=dram_win_ap(out_d, c, wr_l),
                                  in_=out_tok[32 * wr_l:32 * wr_l + 32, :, :])

        for p in reversed(pools):
            p.release()

    _split_multiwaits(nc)
    return nc, consts


def kernel(x, w_qkv, w_out, b_out, rel_bias, ln1_g, ln1_b, ln2_g, ln2_b,
           w1, b1, w2, b2):
    from concourse.bass_utils import run_bass_kernel_spmd

    if "nc" not in _CTX:
        weights = dict(w_qkv=w_qkv, w_out=w_out, b_out=b_out,
                       rel_bias=rel_bias, ln1_g=ln1_g, ln1_b=ln1_b,
                       ln2_g=ln2_g, ln2_b=ln2_b, w1=w1, b1=b1, w2=w2, b2=b2)
        nc, consts = _build(weights)
        _CTX["nc"] = nc
        _CTX["consts"] = consts

    x = np.asarray(x, np.float32)
    in_maps = []
    for i in range(B):
        m = {"x": np.ascontiguousarray(x[i])}
        m.update(_CTX["consts"])
        in_maps.append(m)
    _CTX["in_maps"] = in_maps

    res = run_bass_kernel_spmd(_CTX["nc"], in_maps, core_ids=list(range(B)))
    global LAST_EXEC_NS
    if res.exec_time_ns:
        LAST_EXEC_NS = res.exec_time_ns
    out = np.stack([np.asarray(res.results[i]["out"]) for i in range(B)])
    return out.astype(np.float32)


# revision 7
# speedup vs baseline: 1.2502x; 1.0830x over previous
"""Trainium2 Bass kernel for nn_AttentionBlock (Swin-style 7x7 window attention).

Sharding: pure data parallel - batch B=8, one image per NeuronCore; small
weights and the 169x4 relative-bias table replicated (host-folded).

Per-core program (one image, built with Bass/Tile):
- Token order: window-compact, s = 49*w + p; chunk = 128 windows = 6272 tokens.
- Phase A: LN1 (token-major, window-on-partition) -> PE-transpose ->
  feature-major QKV (bf16 matmuls) -> per-window scores with PSUM-accumulated
  relative bias (pad columns = -1e30 so exp()=0) -> ACT exp -> AV matmuls with
  a ones-augmented V (sumexp lands per-token) -> normalize (DVE reciprocal +
  broadcast multiply) -> PE-transpose -> w_out -> residual -> y to DRAM.
- Phase B: LN2 -> FFN (gelu on ACT) -> residual -> scatter back to image order.

Self-contained: shapes/strategy hardcoded; only library imports.
"""
import numpy as np
import ml_dtypes

_CTX = {}
LAST_EXEC_NS = None

B = 8
N = 50176
D = 96
H = 4
DH = 32
HID = 384
EPS = 1e-5
W = 7
SCALE = DH ** -0.5
NEG = -1e30
NCHUNK = 8


def _rel_idx():
    pos = np.arange(W)
    gi, gj = np.meshgrid(pos, pos, indexing="ij")
    grid = np.stack([gi, gj], -1).reshape(-1, 2)
    rel = grid[:, None] - grid[None] + (W - 1)
    return rel[..., 0] * (2 * W - 1) + rel[..., 1]


def _host_consts(w_qkv, w_out, b_out, rel_bias, ln1_g, ln1_b, ln2_g, ln2_b,
                 w1, b1, w2, b2):
    bf = ml_dtypes.bfloat16
    w_qkv = np.asarray(w_qkv, np.float32)
    wq, wk, wv = w_qkv[0:128], w_qkv[128:256], w_qkv[256:384]
    g1 = np.asarray(ln1_g, np.float32)
    b1v = np.asarray(ln1_b, np.float32)

    def aug(wmat, gamma, beta, extra_scale=1.0):
        out = np.zeros((97, wmat.shape[0]), np.float32)
        out[0:96] = (wmat * gamma[None, :] * extra_scale).T
        out[96] = (wmat * extra_scale) @ beta
        return out

    wqT = aug(wq, g1, b1v, SCALE)
    wkT = aug(wk, g1, b1v)
    wvT_c = aug(wv, g1, b1v)
    wv_augT = np.zeros((97, 132), np.float32)
    for h in range(H):
        wv_augT[:, 33 * h:33 * h + 32] = wvT_c[:, 32 * h:32 * h + 32]
        wv_augT[96, 33 * h + 32] = 1.0
    rb = np.asarray(rel_bias, np.float32)
    bias_h = rb[_rel_idx()].transpose(2, 0, 1) * SCALE
    bias_tbl = np.full((49, H, 64), NEG, np.float32)
    for h in range(H):
        bias_tbl[:, h, 0:49] = bias_h[h]
    g2 = np.asarray(ln2_g, np.float32)
    b2v = np.asarray(ln2_b, np.float32)
    w1m = np.asarray(w1, np.float32)
    w1_augT = np.zeros((97, HID), np.float32)
    w1_augT[0:96] = (w1m * g2[None, :]).T
    w1_augT[96] = w1m @ b2v + np.asarray(b1, np.float32)
    w2T = np.asarray(w2, np.float32).T.reshape(3, 128, 96).transpose(1, 0, 2)
    c = {
        "wqT": wqT, "wkT": wkT, "wv_augT": wv_augT,
        "bias_tbl": bias_tbl, "eye49": np.eye(49, dtype=np.float32),
        "w_outT": np.asarray(w_out, np.float32).T,
        "b_out": np.asarray(b_out, np.float32).reshape(96, 1),
        "w1_augT": w1_augT, "w2T": w2T,
        "b2": np.asarray(b2, np.float32).reshape(96, 1),
        "ones_row": np.ones((1, 8192), np.float32),
    }
    return {k: (v.astype(np.float32) if k in ("b_out", "b2") else v.astype(bf))
            for k, v in c.items()}


def _split_multiwaits(nc, max_waits=1):
    """Walrus here allows 1 sync-wait per instruction; Tile emits multi-wait
    instructions. Split extras onto same-engine nops inserted just before."""
    import bass_rust

    def make_nop(eng):
        if hasattr(eng, "nop"):
            try:
                bi = eng.nop()
                return bi.ins if hasattr(bi, "ins") else bi
            except (AttributeError, TypeError):
                pass
        bi = eng.engine_nop()
        return bi.ins if hasattr(bi, "ins") else bi

    engines = {}
    for name in ("tensor", "vector", "scalar", "gpsimd", "sync"):
        eng = getattr(nc, name)
        engines[eng.engine] = eng
    n_split = 0
    for bbname, bbw in list(nc.bb_map.items()):
        lst = bbw.bb.instructions
        k = 0
        while k < len(lst):
            inst = lst[k]
            si = inst.sync_info
            if si is None:
                k += 1
                continue
            waits = list(si.on_wait or [])
            if len(waits) > max_waits:
                eng = engines.get(inst.engine)
                if eng is None:
                    k += 1
                    continue
                extra, keep = waits[:-max_waits], waits[-max_waits:]
                nops = []
                for w in extra:
                    nop_inst = make_nop(eng)
                    for obbw in nc.bb_map.values():
                        ol = obbw.bb.instructions
                        removed = False
                        for j in range(len(ol) - 1, -1, -1):
                            if ol[j] is nop_inst:
                                ol.pop(j)
                                removed = True
                                break
                        if removed:
                            break
                    nop_inst.sync_info = bass_rust.SyncInfo(on_wait=[w],
                                                            on_update=[])
                    nops.append(nop_inst)
                si.on_wait = keep
                inst.sync_info = si
                for j, nop_inst in enumerate(nops):
                    lst.insert(k + j, nop_inst)
                k += len(nops)
                n_split += 1
            k += 1
    return n_split


def _build(weights, n_chunks=NCHUNK):
    import concourse.bass as bass
    import concourse.tile as tile
    from concourse import mybir
    from concourse.masks import make_identity

    F32 = mybir.dt.float32
    BF16 = mybir.dt.bfloat16
    AF = mybir.ActivationFunctionType
    OP = mybir.AluOpType
    AX = mybir.AxisListType

    consts = _host_consts(**weights)
    nc = bass.Bass()
    xin = nc.declare_dram_parameter("x", [N, D], F32, isOutput=False)
    out_d = nc.declare_dram_parameter("out", [N, D], F32, isOutput=True)
    y_d = nc.dram_tensor("y_buf", [1024, 49, D], F32)
    cd = {}
    for k, v in consts.items():
        dt = F32 if v.dtype == np.float32 else BF16
        cd[k] = nc.declare_dram_parameter(k, list(v.shape), dt, isOutput=False)

    def dram_win_ap(t, chunk, wr_l):
        wr = 4 * chunk + wr_l
        base = (224 * 7 * wr) * 96
        return bass.AP(tensor=t, offset=base,
                       ap=[[7 * 96, 32], [224 * 96, 7], [1, 7 * 96]])

    with tile.TileContext(nc) as tc:
        pools = []

        def pool(name, bufs, space="SBUF"):
            p = tc.alloc_tile_pool(name=name, bufs=bufs, space=space)
            pools.append(p)
            return p

        kp = pool("konst", 1)
        sb_c = {}
        for k, darr in cd.items():
            if k == "ones_row":
                continue
            t = kp.tile(list(darr.shape), darr.dtype, name=f"c_{k}",
                        tag=f"c_{k}")
            nc.sync.dma_start(out=t,
                              in_=darr[(slice(None),) * len(darr.shape)])
            sb_c[k] = t
        ident = kp.tile([128, 128], BF16, name="ident", tag="ident")
        make_identity(nc, ident)

        xp = pool("xp", 2)
        statp = pool("statp", 2)
        lnp = pool("lnp", 1)
        ltp = pool("ltp", 1)
        qkp = pool("qkp", 1)
        vap = pool("vap", 6)
        exp_p = pool("exp_p", 2)
        onp = pool("onp", 2)
        otp = pool("otp", 2)
        aop = pool("aop", 1)
        yp = pool("yp", 1)
        tokp = pool("tokp", 1)
        h1p = pool("h1p", 3)
        outp = pool("outp", 1)

        ps_s = pool("ps_s", 2, space="PSUM")
        ps_a = pool("ps_a", 2, space="PSUM")
        ps_m = pool("ps_m", 2, space="PSUM")

        def ln_layer(src, name):
            """src [128, 49, 96] f32 -> lnt [128, 49, 128] bf16 (pad cols junk).

            Batched stats: one reduce for sums, one mul + reduce for sumsq.
            """
            sums = statp.tile([128, 49], F32, name=f"sm_{name}", tag="sums")
            nc.vector.tensor_reduce(out=sums, in_=src, axis=AX.X, op=OP.add)
            sq = lnp.tile([128, 49, 96], F32, name=f"sq_{name}", tag="lnt")
            nc.vector.tensor_mul(sq, src, src)
            sumsq = statp.tile([128, 49], F32, name=f"s2_{name}", tag="sumsq")
            nc.vector.tensor_reduce(out=sumsq, in_=sq, axis=AX.X, op=OP.add)
            m = statp.tile([128, 49], F32, name=f"m_{name}", tag="m")
            nc.vector.tensor_scalar_mul(m, sums, 1.0 / 96)
            # veps = sumsq/96 - m^2 + EPS
            m2 = statp.tile([128, 49], F32, name=f"m2_{name}", tag="m2")
            nc.vector.tensor_mul(m2, m, m)
            veps = statp.tile([128, 49], F32, name=f"ve_{name}", tag="veps")
            nc.vector.scalar_tensor_tensor(
                out=veps, in0=sumsq, scalar=1.0 / 96, op0=OP.mult,
                in1=m2, op1=OP.subtract)
            nc.vector.tensor_scalar_add(veps, veps, EPS)
            lnv = statp.tile([128, 49], F32, name=f"lv_{name}", tag="lnv")
            nc.scalar.activation(out=lnv, in_=veps, func=AF.Ln)
            rstd = statp.tile([128, 49], F32, name=f"rs_{name}", tag="rstd")
            nc.scalar.activation(out=rstd, in_=lnv, func=AF.Exp, scale=-0.5)
            # x - m (broadcast), reusing the x^2 scratch, then * rstd -> bf16
            nc.vector.scalar_tensor_tensor(
                out=sq, in0=src, scalar=1.0, op0=OP.mult,
                in1=m[:, :, None].to_broadcast([128, 49, 96]),
                op1=OP.subtract)
            lnt = lnp.tile([128, 49, 128], BF16, name=f"ln_{name}", tag="lnt2")
            nc.vector.scalar_tensor_tensor(
                out=lnt[:, :, 0:96], in0=sq, scalar=1.0, op0=OP.mult,
                in1=rstd[:, :, None].to_broadcast([128, 49, 96]),
                op1=OP.mult)
            return lnt

        def transpose_to_T(lnt, dstT):
            """DMA block-transpose: lnt [128, 49, 128] -> dstT[:, 0:6272]
            position-major (col = 128*p + w)."""
            nc.sync.dma_start_transpose(
                dstT[:, 0:6272].rearrange("a (p w) -> a p w", w=128),
                lnt.rearrange("a b c -> a (b c)"))

        def transpose_to_tok(srcT, dst_tok):
            """DMA block-transpose: srcT [96, 6272] -> dst_tok [128, 49, 96]."""
            nc.sync.dma_start_transpose(dst_tok, srcT[0:96, 0:6272])

        eye_b = sb_c["eye49"][:, None, :].to_broadcast([49, 8, 49])

        # ---------------- PHASE A ----------------
        for c in range(n_chunks):
            x_tok = xp.tile([128, 49, 96], F32, name="x_tok", tag="x_tok")
            for wr_l in range(4):
                nc.sync.dma_start(out=x_tok[32 * wr_l:32 * wr_l + 32, :, :],
                                  in_=dram_win_ap(xin, c, wr_l))

            ln_x = ln_layer(x_tok, f"a{c}")
            ln_xT = ltp.tile([128, 8192], BF16, name="ln_xT", tag="ln_xT")
            nc.gpsimd.memset(ln_xT[0:97, 6272:8192], 0.0)
            transpose_to_T(ln_x, ln_xT)
            nc.sync.dma_start(out=ln_xT[96:97, 0:6272],
                              in_=cd["ones_row"][0:1, 0:6272])
            ln_xT_w = ln_xT.rearrange("k (p w) -> k w p", w=128)

            qT = qkp.tile([128, 6272], BF16, name="qT", tag="qT")
            kT = qkp.tile([128, 6272], BF16, name="kT", tag="kT")
            for n0 in range(0, 6272, 448):
                qps = ps_m.tile([128, 448], F32, name="qps", tag="mm")
                nc.tensor.matmul(qps, sb_c["wqT"][0:97, :],
                                 ln_xT[0:97, n0:n0 + 448],
                                 start=True, stop=True)
                nc.vector.tensor_copy(out=qT[:, n0:n0 + 448], in_=qps)
                kps = ps_m.tile([128, 448], F32, name="kps", tag="mm")
                nc.tensor.matmul(kps, sb_c["wkT"][0:97, :],
                                 ln_xT[0:97, n0:n0 + 448],
                                 start=True, stop=True)
                nc.scalar.copy(out=kT[:, n0:n0 + 448], in_=kps)

            qT_w = qT.rearrange("k (p w) -> k w p", w=128)
            kT_w = kT.rearrange("k (p w) -> k w p", w=128)

            attn_T = aop.tile([96, 6272], BF16, name="attn_T", tag="attn_T")
            attn_T_w = attn_T.rearrange("a (p w) -> a p w", w=128)

            for g in range(8):
                w0 = 16 * g
                v_tiles = []
                for q2 in range(4):
                    vps = ps_a.tile([128, 264], F32, name="vps", tag="av")
                    for dpq in range(2):
                        for par in range(2):
                            wloc = w0 + 2 * (2 * q2 + dpq) + par
                            nc.tensor.matmul(
                                vps[64 * par:64 * par + 64,
                                    132 * dpq:132 * dpq + 132],
                                ln_xT_w[0:97, wloc, :],
                                sb_c["wv_augT"][0:97, :],
                                start=True, stop=True, skip_group_check=True)
                    v_sb = vap.tile([128, 264], BF16, name="v_sb", tag="v_sb")
                    nc.scalar.copy(out=v_sb, in_=vps)
                    v_tiles.append(v_sb)

                exp_ts = []
                for hh in (0, 2):
                    sps = ps_s.tile([128, 1024], F32, name="sps", tag="sps")
                    sps_r = sps.rearrange("a (r x) -> a r x", x=512)
                    for r in range(2):
                        h = hh + r
                        for par in range(2):
                            nc.tensor.matmul(
                                sps_r[64 * par:64 * par + 64, r, 0:392],
                                sb_c["bias_tbl"][:, h, :], eye_b,
                                start=True, stop=False, skip_group_check=True)
                        for j in range(8):
                            for par in range(2):
                                wloc = w0 + 2 * j + par
                                nc.tensor.matmul(
                                    sps_r[64 * par:64 * par + 49, r,
                                          49 * j:49 * j + 49],
                                    kT_w[32 * h:32 * h + 32, wloc, 0:49],
                                    qT_w[32 * h:32 * h + 32, wloc, 0:49],
                                    start=False, stop=True,
                                    tile_position=(32 * h, 64 * par),
                                    skip_group_check=True)
                    exp_sb = exp_p.tile([128, 2, 392], BF16, name="exp_sb",
                                        tag="exp_sb")
                    nc.scalar.activation(out=exp_sb, in_=sps_r[:, :, 0:392],
                                         func=AF.Exp)
                    exp_ts.append(exp_sb)

                o_norm_g = onp.tile([128, 1024], BF16, name="o_norm_g",
                                    tag="o_norm")
                for q2 in range(4):
                    avp = ps_a.tile([128, 264], F32, name="avp", tag="av")
                    avp_b = avp.rearrange("a (b c) -> a b c", c=33)
                    for dpq in range(2):
                        j = 2 * q2 + dpq
                        for h in range(4):
                            e_sb = exp_ts[h // 2]
                            r = h % 2
                            for par in range(2):
                                nc.tensor.matmul(
                                    avp[64 * par:64 * par + 49,
                                        132 * dpq + 33 * h:
                                        132 * dpq + 33 * h + 33],
                                    e_sb[64 * par:64 * par + 64, r,
                                         49 * j:49 * j + 49],
                                    v_tiles[q2][64 * par:64 * par + 64,
                                                132 * dpq + 33 * h:
                                                132 * dpq + 33 * h + 33],
                                    start=True, stop=True,
                                    skip_group_check=True)
                    rc = statp.tile([128, 8], F32, name="rc", tag="rc")
                    nc.vector.reciprocal(
                        rc.rearrange("a (b c) -> a b c", c=1),
                        avp_b[:, :, 32:33])
                    nc.vector.scalar_tensor_tensor(
                        out=o_norm_g[:, 256 * q2:256 * q2 + 256].rearrange(
                            "a (b c) -> a b c", c=32),
                        in0=avp_b[:, :, 0:32], scalar=1.0, op0=OP.mult,
                        in1=rc[:, :, None].to_broadcast([128, 8, 32]),
                        op1=OP.mult)

                # one block-transpose + two 4-pair w_out matmuls per group
                oT_sb = otp.tile([128, 1024], BF16, name="oT_sb", tag="oT_sb")
                nc.sync.dma_start_transpose(
                    oT_sb.rearrange("a (r c) -> a r c", c=128),
                    o_norm_g[:, :])
                for half in range(2):
                    wop = ps_m.tile([96, 392], F32, name="wop", tag="mm")
                    nc.tensor.matmul(
                        wop, sb_c["w_outT"],
                        oT_sb.rearrange("a (r m q) -> a r m q",
                                        r=8, q=64)[:, 4 * half:4 * half + 4,
                                                   :, 0:49],
                        start=True, stop=True)
                    wbase = 2 * (8 * g + 4 * half)
                    nc.vector.tensor_scalar_add(
                        attn_T_w[:, :, wbase:wbase + 8].transpose([0, 2, 1]),
                        wop.rearrange("a (r m q) -> a (r m) q", r=4, m=2),
                        sb_c["b_out"][0:96, 0:1])

            attn_tok = tokp.tile([128, 49, 96], BF16, name="attn_tok",
                                 tag="tok_b")
            transpose_to_tok(attn_T, attn_tok)
            y_tok = yp.tile([128, 49, 96], F32, name="y_tok", tag="y_tok")
            nc.vector.scalar_tensor_tensor(
                out=y_tok, in0=attn_tok, scalar=1.0, op0=OP.mult,
                in1=x_tok, op1=OP.add)
            nc.sync.dma_start(out=y_d[128 * c:128 * c + 128, :, :], in_=y_tok)

        # ---------------- PHASE B ----------------
        for c in range(n_chunks):
            y_in = xp.tile([128, 49, 96], F32, name="y_in", tag="x_tok")
            nc.sync.dma_start(out=y_in, in_=y_d[128 * c:128 * c + 128, :, :])

            ln2 = ln_layer(y_in, f"b{c}")
            ln2T = ltp.tile([128, 6272], BF16, name="ln2T", tag="ln_xT")
            transpose_to_T(ln2, ln2T)
            nc.sync.dma_start(out=ln2T[96:97, 0:6272],
                              in_=cd["ones_row"][0:1, 0:6272])

            ffn_T = aop.tile([96, 6272], BF16, name="ffn_T", tag="attn_T")
            for n0 in range(0, 6272, 448):
                h1 = h1p.tile([128, 3, 448], BF16, name="h1", tag="h1")
                for m in range(3):
                    fps = ps_m.tile([128, 448], F32, name="fps", tag="mm")
                    nc.tensor.matmul(
                        fps, sb_c["w1_augT"][0:97, 128 * m:128 * m + 128],
                        ln2T[0:97, n0:n0 + 448], start=True, stop=True)
                    nc.scalar.activation(out=h1[:, m, :], in_=fps,
                                         func=AF.Gelu)
                f2 = ps_m.tile([96, 448], F32, name="f2", tag="mm")
                for m in range(3):
                    nc.tensor.matmul(f2, sb_c["w2T"][:, m, :], h1[:, m, :],
                                     start=(m == 0), stop=(m == 2),
                                     skip_group_check=True)
                nc.vector.tensor_scalar_add(ffn_T[0:96, n0:n0 + 448], f2,
                                             sb_c["b2"][0:96, 0:1])

            ffn_tok = tokp.tile([128, 49, 96], BF16, name="ffn_tok",
                                tag="tok_b")
            transpose_to_tok(ffn_T, ffn_tok)
            out_tok = outp.tile([128, 49, 96], F32, name="out_tok",
                                tag="out_tok")
            nc.vector.scalar_tensor_tensor(
                out=out_tok, in0=ffn_tok, scalar=1.0, op0=OP.mult,
                in1=y_in, op1=OP.add)
            for wr_l in range(4):
                nc.sync.dma_start(out=dram_win_ap(out_d, c, wr_l),
                                  in_=out_tok[32 * wr_l:32 * wr_l + 32, :, :])

        for p in reversed(pools):
            p.release()

    _split_multiwaits(nc)
    return nc, consts


def kernel(x, w_qkv, w_out, b_out, rel_bias, ln1_g, ln1_b, ln2_g, ln2_b,
           w1, b1, w2, b2):
    from concourse.bass_utils import run_bass_kernel_spmd

    if "nc" not in _CTX:
        weights = dict(w_qkv=w_qkv, w_out=w_out, b_out=b_out,
                       rel_bias=rel_bias, ln1_g=ln1_g, ln1_b=ln1_b,
                       ln2_g=ln2_g, ln2_b=ln2_b, w1=w1, b1=b1, w2=w2, b2=b2)
        nc, consts = _build(weights)
        _CTX["nc"] = nc
        _CTX["consts"] = consts

    x = np.asarray(x, np.float32)
    in_maps = []
    for i in range(B):
        m = {"x": np.ascontiguousarray(x[i])}
        m.update(_CTX["consts"])
        in_maps.append(m)
    _CTX["in_maps"] = in_maps

    res = run_bass_kernel_spmd(_CTX["nc"], in_maps, core_ids=list(range(B)))
    global LAST_EXEC_NS
    if res.exec_time_ns:
        LAST_EXEC_NS = res.exec_time_ns
    out = np.stack([np.asarray(res.results[i]["out"]) for i in range(B)])
    return out.astype(np.float32)
